# revision 6
# baseline (speedup 1.0000x reference)
"""Trainium2 Bass kernel v2 for the dense attention block.

Reference (per batch b):
    qkv = x @ w_qkv ; per head h: attn = softmax(q_h k_h^T / sqrt(128))
    out = concat_h(attn @ v_h) @ w_out

Sharding: 2-way data-parallel (batch) x 4-way head-parallel (4 heads/core).
Each core emits a partial out [S, DOUT] (bf16); host sums per batch.

v2 design vs baseline:
  - bf16 inputs/weights/activations everywhere (fp32 PSUM accumulate):
    halves DMA + LDWEIGHTS time; accuracy ~6e-3 rel (tol 2e-2).
  - x loaded once (resident in SBUF), V+QK share it.
  - phase order QK(h0) -> V -> [attn(h) + QK(h+1) interleaved] -> h3 also
    interleaves the out-projection: Act exp work hides under Tensor work.
  - rowsum via ones-matmul with 128-wide ones stationary -> PSUM [128,512]
    holds the rowsum broadcast across partitions: kills the DRAM-bounce
    broadcast and the slow [1,512] reciprocal of the baseline.
  - optional fp8e4 DoubleRow rowsum (halves rowsum matmul instructions).
"""

import numpy as np
import ml_dtypes

B, S, DIM = 2, 2048, 2048
NUM_HEADS, HEAD_DIM = 16, 128
N_CORES = 8
HEAD_SHARDS = 4
NH = NUM_HEADS // HEAD_SHARDS   # 4 heads per core

EXP_BIAS = 2.0                  # exp(logits*scale - EXP_BIAS); cancels in softmax
ROWSUM_FP8 = True              # fp8e4 DoubleRow rowsum (else bf16 ones-matmul)


def build_nc(rowsum_fp8=ROWSUM_FP8):
    import concourse.bacc as bacc
    import concourse.mybir as mybir
    import concourse.tile as tile
    from contextlib import ExitStack

    fp32 = mybir.dt.float32
    bf16 = mybir.dt.bfloat16
    fp8 = mybir.dt.float8e4
    DRmode = mybir.MatmulPerfMode.DoubleRow
    P = 128
    FREE = 512
    n_t = DIM // P            # 16 contraction tiles
    n_s4 = S // FREE          # 4 token slices (512)
    n_sk = S // P             # 16 key tiles
    n_dm = DIM // FREE        # 4 out-col slices
    scale = 1.0 / (HEAD_DIM ** 0.5)
    Exp = mybir.ActivationFunctionType.Exp

    nc = bacc.Bacc("TRN2")
    xT = nc.dram_tensor("xT", [DIM, S], bf16, kind="ExternalInput")
    w_qkv = nc.dram_tensor("w_qkv", [DIM, 3 * NH * HEAD_DIM], bf16,
                           kind="ExternalInput")
    w_out = nc.dram_tensor("w_out", [NH * HEAD_DIM, DIM], bf16,
                           kind="ExternalInput")
    onesb_in = nc.dram_tensor("onesb", [P, P], bf16, kind="ExternalInput")
    ones8_in = nc.dram_tensor("ones8", [P, 2, P], fp8, kind="ExternalInput")
    out = nc.dram_tensor("out", [S, DIM], bf16, kind="ExternalOutput")

    mm = nc.tensor.matmul

    with tile.TileContext(nc) as tc, ExitStack() as ctx:
        persist = ctx.enter_context(tc.tile_pool(name="persist", bufs=1))
        onesb = persist.tile([P, P], bf16, tag="onesb")
        ones8 = persist.tile([P, 2, P], fp8, tag="ones8")
        bias_ap = persist.tile([P, 1], fp32, tag="bias")
        nc.vector.memset(bias_ap, -float(EXP_BIAS))

        # ---- persistent SBUF tensors -----------------------------------
        xpool = ctx.enter_context(tc.tile_pool(name="x", bufs=1))
        x_all = xpool.tile([P, n_t, S], bf16, tag="x")
        xt = [x_all[:, t, :] for t in range(n_t)]
        wvpool = ctx.enter_context(tc.tile_pool(name="wv", bufs=1))
        wv = wvpool.tile([P, n_t, FREE], bf16, tag="wv")
        wqpool = ctx.enter_context(tc.tile_pool(name="wq", bufs=2))
        wq = {}
        wk = {}
        qkpool = ctx.enter_context(tc.tile_pool(name="qkT", bufs=1))
        qT = [qkpool.tile([P, S], bf16, tag=f"qT{h}", name=f"qT{h}")
              for h in range(NH)]
        kT = [qkpool.tile([P, S], bf16, tag=f"kT{h}", name=f"kT{h}")
              for h in range(NH)]
        v4pool = ctx.enter_context(tc.tile_pool(name="v4", bufs=1))
        v4 = [v4pool.tile([P, FREE], bf16, tag=f"v4_{i}", name=f"v4_{i}")
              for i in range(n_sk)]
        combpool = ctx.enter_context(tc.tile_pool(name="comb", bufs=1))
        comb = [combpool.tile([P, S], bf16, tag=f"comb{h}", name=f"comb{h}")
                for h in range(NH)]
        wopool = ctx.enter_context(tc.tile_pool(name="wo", bufs=1))
        wo = [wopool.tile([P, DIM], bf16, tag=f"wo{h}", name=f"wo{h}")
              for h in range(NH)]

        def load_wqk(h):
            wq[h] = wqpool.tile([P, n_t, P], bf16, tag="wq", name=f"wq{h}")
            wk[h] = wqpool.tile([P, n_t, P], bf16, tag="wk", name=f"wk{h}")
            nc.sync.dma_start(
                out=wq[h],
                in_=w_qkv[:, h * P:(h + 1) * P].rearrange("(t p) c -> p t c", p=P))
            nc.sync.dma_start(
                out=wk[h],
                in_=w_qkv[:, NH * P + h * P:NH * P + (h + 1) * P].rearrange(
                    "(t p) c -> p t c", p=P))

        # DMA order tuned for fastest first-matmul: chunked wq0/x0/wk0
        # first, constants (ones/wv) deferred behind the early x tiles.
        xT_r = xT.rearrange("(t p) s -> p t s", p=P)
        wq0_d = w_qkv[:, 0:P].rearrange("(t p) c -> p t c", p=P)
        wk0_d = w_qkv[:, NH * P:(NH + 1) * P].rearrange("(t p) c -> p t c", p=P)
        wq[0] = wqpool.tile([P, n_t, P], bf16, tag="wq", name="wq0")
        wk[0] = wqpool.tile([P, n_t, P], bf16, tag="wk", name="wk0")
        HALF = 2 * FREE
        nc.sync.dma_start(out=wq[0][:, 0:2, :], in_=wq0_d[:, 0:2, :])
        nc.sync.dma_start(out=x_all[:, 0, 0:FREE], in_=xT_r[:, 0, 0:FREE])
        nc.sync.dma_start(out=wk[0][:, 0:2, :], in_=wk0_d[:, 0:2, :])
        nc.sync.dma_start(out=x_all[:, 0, FREE:HALF], in_=xT_r[:, 0, FREE:HALF])
        nc.sync.dma_start(out=wq[0][:, 2:4, :], in_=wq0_d[:, 2:4, :])
        nc.sync.dma_start(out=wk[0][:, 2:4, :], in_=wk0_d[:, 2:4, :])
        nc.sync.dma_start(out=x_all[:, 1, 0:HALF], in_=xT_r[:, 1, 0:HALF])
        nc.sync.dma_start(out=wq[0][:, 4:, :], in_=wq0_d[:, 4:, :])
        nc.sync.dma_start(out=wk[0][:, 4:, :], in_=wk0_d[:, 4:, :])
        for t in range(2, n_t):
            nc.sync.dma_start(out=x_all[:, t, 0:HALF], in_=xT_r[:, t, 0:HALF])
        for t in range(n_t):
            nc.sync.dma_start(out=x_all[:, t, HALF:], in_=xT_r[:, t, HALF:])
            if t == 1:
                nc.sync.dma_start(out=onesb, in_=onesb_in[:, :])
                nc.sync.dma_start(out=ones8, in_=ones8_in[:, :, :])
        nc.sync.dma_start(
            out=wv, in_=w_qkv[:, 2 * NH * P:3 * NH * P].rearrange(
                "(t p) c -> p t c", p=P))

        # ---- QK(h0) two-pass over 4 reused PSUM banks + V sharing the
        # same scope (psv coexists; V never waits on QK0's copies) --------
        with tc.tile_pool(name="psqk0", bufs=1, space="PSUM") as psqk0, \
             tc.tile_pool(name="psv", bufs=3, space="PSUM") as psv:
            for s4pair in range(2):
                ps = {}
                for f in range(2):
                    for s4 in (2 * s4pair, 2 * s4pair + 1):
                        ps[f, s4] = psqk0.tile(
                            [P, FREE], fp32, tag=f"q0_{f}_{s4 % 2}",
                            name=f"q0_{f}_{s4}")
                for t in range(n_t):
                    for f in range(2):
                        w = wq[0] if f == 0 else wk[0]
                        dst = qT[0] if f == 0 else kT[0]
                        for s4 in (2 * s4pair, 2 * s4pair + 1):
                            mm(ps[f, s4], w[:, t, :],
                               xt[t][:, s4 * FREE:(s4 + 1) * FREE],
                               start=(t == 0), stop=(t == n_t - 1))
                            if t == n_t - 1:
                                nc.vector.tensor_copy(
                                    dst[:, s4 * FREE:(s4 + 1) * FREE],
                                    ps[f, s4])
            load_wqk(1)
            for c in range(n_sk):
                pv = psv.tile([P, FREE], fp32, tag="pv")
                for t in range(n_t):
                    mm(pv, xt[t][:, c * P:(c + 1) * P], wv[:, t, :],
                       start=(t == 0), stop=(t == n_t - 1))
                nc.vector.tensor_copy(v4[c], pv)

        # ---- interleaved attention + next-head QK + out-projection -----
        def qk_chain(h, f, s4, pool):
            """one (feature, s4) projection chain for head h (16 matmuls)"""
            w = wq[h] if f == 0 else wk[h]
            dst = qT[h] if f == 0 else kT[h]
            pq = pool.tile([P, FREE], fp32, tag="aux")
            for t in range(n_t):
                mm(pq, w[:, t, :], xt[t][:, s4 * FREE:(s4 + 1) * FREE],
                   start=(t == 0), stop=(t == n_t - 1))
            nc.vector.tensor_copy(dst[:, s4 * FREE:(s4 + 1) * FREE], pq)

        def out_proj(sq4, pools, last=False):
            opool, psout = pools
            for tok in range(sq4 * n_s4, (sq4 + 1) * n_s4):
                tk = slice(tok * P, (tok + 1) * P)
                ot = opool.tile([P, DIM], bf16, tag="ot")
                chunked = last and tok == (sq4 + 1) * n_s4 - 1
                for dm in range(n_dm):
                    dms = slice(dm * FREE, (dm + 1) * FREE)
                    pso = psout.tile([P, FREE], fp32, tag="aux")
                    for hd in range(NH):
                        mm(pso, comb[hd][:, tk], wo[hd][:, dms],
                           start=(hd == 0), stop=(hd == NH - 1))
                    if chunked and dm % 2 == 0:
                        nc.scalar.copy(ot[:, dms], pso)
                    else:
                        nc.vector.tensor_copy(ot[:, dms], pso)
                    if chunked:
                        nc.sync.dma_start(out[tk, dms], ot[:, dms])
                if not chunked:
                    nc.sync.dma_start(out[tk, :], ot)

        with tc.tile_pool(name="et", bufs=8) as epool, \
             tc.tile_pool(name="et8", bufs=3) as e8pool, \
             tc.tile_pool(name="rc", bufs=2) as rcpool, \
             tc.tile_pool(name="pslg", bufs=3, space="PSUM") as pslg, \
             tc.tile_pool(name="psav", bufs=1, space="PSUM") as psav, \
             tc.tile_pool(name="psrs", bufs=1, space="PSUM") as psrs, \
             tc.tile_pool(name="psaux", bufs=3, space="PSUM") as psaux, \
             tc.tile_pool(name="ot", bufs=2) as opool:
            for h in range(NH):
                if h + 2 < NH:
                    load_wqk(h + 2)
                if h == 2:
                    for hd in range(NH):
                        nc.sync.dma_start(
                            out=wo[hd], in_=w_out[hd * P:(hd + 1) * P, :])
                for sq4 in range(n_s4):
                    sq = slice(sq4 * FREE, (sq4 + 1) * FREE)
                    ps_av = psav.tile([P, FREE], fp32, tag="av")
                    ps_rs = psrs.tile([P, FREE], fp32, tag="rs")
                    for skt in range(n_sk):
                        ps_lg = pslg.tile([P, FREE], fp32, tag="lg")
                        mm(ps_lg, kT[h][:, skt * P:(skt + 1) * P], qT[h][:, sq],
                           start=True, stop=True)
                        et = epool.tile([P, FREE], bf16, tag="et")
                        nc.scalar.activation(out=et, in_=ps_lg, func=Exp,
                                             scale=scale, bias=bias_ap)
                        if rowsum_fp8:
                            if skt % 2 == 0:
                                e8 = e8pool.tile([P, 2, FREE], fp8, tag="e8")
                            nc.vector.tensor_copy(e8[:, skt % 2, :], et)
                            if skt % 2 == 1:
                                mm(ps_rs, ones8, e8, start=(skt == 1),
                                   stop=(skt == n_sk - 1), perf_mode=DRmode)
                        else:
                            mm(ps_rs, onesb, et, start=(skt == 0),
                               stop=(skt == n_sk - 1))
                        mm(ps_av, v4[skt][:, h * P:(h + 1) * P], et,
                           start=(skt == 0), stop=(skt == n_sk - 1))
                    rc = rcpool.tile([P, FREE], fp32, tag="rc")
                    nc.vector.reciprocal_approx_fast(out=rc, in_=ps_rs)
                    nc.vector.tensor_mul(comb[h][:, sq], ps_av, rc)
                    # interleave: 2 QK chains of head h+1 per sq4 block
                    if h + 1 < NH:
                        for i in range(2):
                            ci = sq4 * 2 + i
                            qk_chain(h + 1, ci // n_s4, ci % n_s4, psaux)
                    # out-projection shifted one block behind attn(h3)
                    if h == NH - 1 and sq4 > 0:
                        out_proj(sq4 - 1, (opool, psaux))
            out_proj(n_s4 - 1, (opool, psaux), last=True)

    nc.compile()
    return nc


def make_in_maps(x, w_qkv, w_out):
    x = np.asarray(x, dtype=np.float32)
    w_qkv = np.asarray(w_qkv, dtype=np.float32)
    w_out = np.asarray(w_out, dtype=np.float32)
    cols = NH * HEAD_DIM              # 512
    onesb = np.ones((128, 128), dtype=ml_dtypes.bfloat16)
    ones8 = np.ones((128, 2, 128), dtype=ml_dtypes.float8_e4m3)
    in_maps = []
    for c in range(N_CORES):
        b = c // HEAD_SHARDS
        hg = c % HEAD_SHARDS
        q = w_qkv[:, hg * cols:(hg + 1) * cols]
        k = w_qkv[:, NUM_HEADS * HEAD_DIM + hg * cols:
                  NUM_HEADS * HEAD_DIM + (hg + 1) * cols]
        v = w_qkv[:, 2 * NUM_HEADS * HEAD_DIM + hg * cols:
                  2 * NUM_HEADS * HEAD_DIM + (hg + 1) * cols]
        in_maps.append({
            "xT": np.ascontiguousarray(x[b].T).astype(ml_dtypes.bfloat16),
            "w_qkv": np.ascontiguousarray(
                np.concatenate([q, k, v], axis=1)).astype(ml_dtypes.bfloat16),
            "w_out": np.ascontiguousarray(
                w_out[hg * cols:(hg + 1) * cols, :]).astype(ml_dtypes.bfloat16),
            "onesb": onesb,
            "ones8": ones8,
        })
    return in_maps


def combine_outputs(partials):
    out = np.zeros((B, S, DIM), dtype=np.float32)
    for c in range(N_CORES):
        out[c // HEAD_SHARDS] += np.asarray(partials[c], dtype=np.float32)
    return out


_NC_CACHE = None


def kernel(x, w_qkv, w_out):
    global _NC_CACHE
    from concourse import bass_utils
    if _NC_CACHE is None:
        _NC_CACHE = build_nc()
    in_maps = make_in_maps(x, w_qkv, w_out)
    res = bass_utils.run_bass_kernel_spmd(
        _NC_CACHE, in_maps, core_ids=list(range(N_CORES)))
    return combine_outputs([r["out"] for r in res.results])


if __name__ == "__main__":
    nc = build_nc()
    print("built ok")
